# revision 22
# baseline (speedup 1.0000x reference)
"""Multi-head causal attention with RoPE on 8 TRN2 NeuronCores.

Sharding: 8 cores = 2 batches x 4 head-groups (4 heads each).
Per-core Bass kernel computes the group-partial output transposed;
host sums group partials and transposes back.

All matmul operands are bf16 (same PE throughput as fp32r, half the
SBUF/DMA footprint), accumulation in fp32 PSUM. Q^T/K^T/V stay
resident in SBUF (no DRAM scratch round-trip). Causal masking uses
a single triangular 128x128 tile: within each diagonal 128x512 score
tile only one 128-col block is partially masked; fully-masked columns
are skipped by slicing the matmul moving dim.
"""

import numpy as np
import ml_dtypes

import concourse.bass as bass  # noqa: F401
import concourse.tile as tile
from concourse import bacc, mybir

B, S, D, H, HD = 2, 2048, 2048, 16, 128
NCORES = 8
G = 4            # head groups
GH = 4           # heads per group
GD = GH * HD     # 512 dims per group
P = 128
NU = S // 512    # 4 query slices
NT = S // P      # 16 key tiles

_f32 = mybir.dt.float32
_bf16 = mybir.dt.bfloat16
_bf = ml_dtypes.bfloat16

_cache = {}


def _build(causal: bool, reps: int = 1, depth: int = 7):
    nc = bacc.Bacc("TRN2", target_bir_lowering=False, debug=False)
    xT = nc.dram_tensor("xT", [D, S], _bf16, kind="ExternalInput").ap()
    wq = nc.dram_tensor("wq", [D, GD], _bf16, kind="ExternalInput").ap()
    wk = nc.dram_tensor("wk", [D, GD], _bf16, kind="ExternalInput").ap()
    wv = nc.dram_tensor("wv", [D, GD], _bf16, kind="ExternalInput").ap()
    wo = nc.dram_tensor("wo", [GD, D], _bf16, kind="ExternalInput").ap()
    cs = nc.dram_tensor("cs", [P, S], _f32, kind="ExternalInput").ap()
    ss = nc.dram_tensor("ss", [P, S], _f32, kind="ExternalInput").ap()
    ones = nc.dram_tensor("ones", [P, P], _bf16, kind="ExternalInput").ap()
    if causal:
        tri = nc.dram_tensor("tri", [P, P], _bf16, kind="ExternalInput").ap()
    else:
        maskf = nc.dram_tensor("maskf", [S, S], _bf16, kind="ExternalInput").ap()
    outT = nc.dram_tensor("outT", [D, S], _bf16, kind="ExternalOutput").ap()

    Exp = mybir.ActivationFunctionType.Exp

    with tile.TileContext(nc) as tc:
      with (
          tc.tile_pool(name="pw", bufs=1) as pw,
          tc.tile_pool(name="pcon", bufs=1) as pcon,
          tc.tile_pool(name="px", bufs=2 if causal else 1) as px,
          tc.tile_pool(name="pqkv", bufs=1) as pqkv,
          tc.tile_pool(name="paot", bufs=2) as paot,
          tc.tile_pool(name="ptmp", bufs=2) as ptmp,
          tc.tile_pool(name="ppt", bufs=8) as ppt,
          tc.tile_pool(name="pds", bufs=6) as pds,
          tc.tile_pool(name="prec", bufs=2) as prec,
          tc.tile_pool(name="pso", bufs=8) as pso,
          tc.tile_pool(name="pmu", bufs=1) as pmu,
          tc.tile_pool(name="psAB", bufs=2, space="PSUM") as psAB,
          tc.tile_pool(name="psB", bufs=4, space="PSUM") as psB,
      ):
        for _rep in range(reps):
            # ---- per-rep input tiles (pools persist; tags rotate buffers)
            wq_s = pw.tile([P, NT * GD], _bf16, tag="wq")
            wk_s = pw.tile([P, NT * GD], _bf16, tag="wk")
            wv_s = pw.tile([P, NT * GD], _bf16, tag="wv")
            wo_s = pw.tile([P, GH * D], _bf16, tag="wo")
            cs_s = pcon.tile([P, S], _f32, tag="cs")
            ss_s = pcon.tile([P, S], _f32, tag="ss")
            ones_s = pcon.tile([P, P], _bf16, tag="ones")
            if causal:
                tri_s = pcon.tile([P, P], _bf16, tag="tri")
                nc.sync.dma_start(tri_s[:], tri[:])
            qt_s = pqkv.tile([P, GH * S], _bf16, tag="qt")
            kt_s = pqkv.tile([P, GH * S], _bf16, tag="kt")
            v_s = pqkv.tile([P, NT * GD], _bf16, tag="v")

            nc.sync.dma_start(cs_s[:], cs[:])
            nc.sync.dma_start(ss_s[:], ss[:])
            nc.sync.dma_start(ones_s[:], ones[:])
            for (w_s, w_d) in ((wq_s, wq), (wk_s, wk), (wv_s, wv)):
                nc.sync.dma_start(
                    w_s[:].rearrange("p (t j) -> p t j", t=NT),
                    w_d.rearrange("(t p) j -> p t j", t=NT))
            nc.sync.dma_start(
                wo_s[:].rearrange("p (d j) -> p d j", d=GH),
                wo.rearrange("(d p) j -> p d j", d=GH))

            # ---- Phase P: Q^T/K^T (RoPE fused) and V projections -> SBUF
            for u in range(NU):
                su = slice(u * 512, (u + 1) * 512)
                xu = px.tile([P, NT * GD], _bf16, tag="xu")
                nc.sync.dma_start(
                    xu[:].rearrange("p (t j) -> p t j", t=NT),
                    xT[:, u * 512:(u + 1) * 512].rearrange("(t p) j -> p t j", t=NT))
                for (w_s, dst) in ((wq_s, qt_s), (wk_s, kt_s)):
                    for dt in range(GH):
                        pq = psB.tile([P, 512], _f32, tag="ps")
                        for t in range(NT):
                            nc.tensor.matmul(
                                pq[:],
                                w_s[:, t * GD + dt * P: t * GD + dt * P + P],
                                xu[:, t * GD:(t + 1) * GD],
                                start=(t == 0), stop=(t == NT - 1))
                        t1 = ptmp.tile([P, 512], _f32, tag="t1")
                        t2 = ptmp.tile([P, 512], _f32, tag="t2")
                        nc.vector.tensor_mul(t1[:], pq[:], cs_s[:, su])
                        nc.vector.tensor_mul(t2[0:64, :], pq[64:P, :], ss_s[0:64, su])
                        nc.vector.tensor_mul(t2[64:P, :], pq[0:64, :], ss_s[64:P, su])
                        nc.vector.tensor_add(
                            dst[:, dt * S + u * 512: dt * S + (u + 1) * 512],
                            t1[:], t2[:])
                for st in range(4):
                    g = 4 * u + st
                    pv = psB.tile([P, GD], _f32, tag="ps")
                    for t in range(NT):
                        nc.tensor.matmul(
                            pv[:],
                            xu[:, t * GD + st * P: t * GD + st * P + P],
                            wv_s[:, t * GD:(t + 1) * GD],
                            start=(t == 0), stop=(t == NT - 1))
                    nc.vector.tensor_copy(v_s[:, g * GD:(g + 1) * GD], pv[:])

            # ---- Phases A (attention) + W (output projection), staggered
            def attn(u, h):
                n_sk = 4 * (u + 1) if causal else NT
                if not causal:
                    mu = mus[u]
                psa = psAB.tile([P, 512], _f32, tag="psa")
                psd = psAB.tile([P, 512], _f32, tag="psd")
                pts = [None] * n_sk
                # denominator: per-tile (sliced) ones-matmuls, except off-diag
                # quads tree-summed on DVE for deep chains (u >= 2)
                useq = False and causal
                nq = (4 * u) // 4 if useq else 0
                ngrp = (nq + 4 + (0 if useq else 4 * u)) if causal else n_sk
                grp = {}      # last tile t -> (group idx, mm_lo, moving tile)
                gsums = {}

                def consume(t):
                    pt, lo = pts[t]
                    nc.tensor.matmul(psa[:, lo:512],
                                     v_s[:, t * GD + h * P: t * GD + (h + 1) * P],
                                     pt[:, lo:512],
                                     start=(t == 0), stop=(t == n_sk - 1))
                    if t in grp:
                        gi, mlo, stile = grp[t]
                        nc.tensor.matmul(psd[:, mlo:512], ones_s[:],
                                         stile[:, mlo:512],
                                         start=(gi == 0), stop=(gi == ngrp - 1))

                for t in range(n_sk):
                    d = t - 4 * u if causal else -1
                    lo = P * d if d > 0 else 0
                    pss = psB.tile([P, 512], _f32, tag="ps")
                    nc.tensor.matmul(
                        pss[:, lo:512],
                        kt_s[:, h * S + t * P: h * S + (t + 1) * P],
                        qt_s[:, h * S + u * 512 + lo: h * S + (u + 1) * 512],
                        start=True, stop=True)
                    if not causal:
                        nc.vector.tensor_add(
                            pss[:], pss[:], mu[:, t * 512:(t + 1) * 512])
                    pt = ppt.tile([P, 512], _bf16, tag="pt")
                    nc.scalar.activation(pt[:, lo:512], pss[:, lo:512], Exp)
                    if causal and d >= 0:
                        nc.gpsimd.tensor_mul(
                            pt[:, lo:lo + P], pt[:, lo:lo + P], tri_s[:])
                    pts[t] = (pt, lo)
                    if useq and d < 0:
                        if t % 4 == 1:
                            ds = pds.tile([P, 512], _bf16, tag="ds", name="ds")
                            gsums[t // 4] = ds
                            nc.vector.tensor_add(ds[:], pts[t - 1][0][:], pt[:])
                        elif t % 4 == 3:
                            ds2 = pds.tile([P, 512], _bf16, tag="ds", name="ds2")
                            nc.vector.tensor_add(ds2[:], pts[t - 1][0][:], pt[:])
                            ds = gsums[t // 4]
                            nc.vector.tensor_add(ds[:], ds[:], ds2[:])
                            grp[t] = (t // 4, 0, ds)
                    elif useq:
                        grp[t] = (nq + d, lo, pt)
                    else:
                        grp[t] = (t, lo, pt)
                    if t >= depth:
                        consume(t - depth)
                for t in range(max(0, n_sk - depth), n_sk):
                    consume(t)
                rec = prec.tile([P, 512], _f32, tag="rec")
                nc.vector.reciprocal(rec[:], psd[:])
                nc.vector.tensor_mul(
                    aots[u][:, h * 512:(h + 1) * 512], psa[:], rec[:])

            def wproj(u, ots):
                for ot in ots:
                    if ot == NT - 2:
                        po2 = psAB.tile([P, 512], _f32, tag="psa", name="po2a")
                    elif ot == NT - 1:
                        po2 = psAB.tile([P, 512], _f32, tag="psd", name="po2d")
                    else:
                        po2 = psB.tile([P, 512], _f32, tag="ps")
                    for dt in range(GH):
                        nc.tensor.matmul(
                            po2[:],
                            wo_s[:, dt * D + ot * P: dt * D + (ot + 1) * P],
                            aots[u][:, dt * 512:(dt + 1) * 512],
                            start=(dt == 0), stop=(dt == GH - 1))
                    so = pso.tile([P, 512], _bf16, tag="so")
                    if ot < 5:
                        nc.scalar.copy(so[:], po2[:])
                    else:
                        nc.vector.tensor_copy(so[:], po2[:])
                    nc.sync.dma_start(
                        outT[ot * P:(ot + 1) * P, u * 512:(u + 1) * 512], so[:])

            aots = {}
            mus = {}
            for u in range(NU):
                aots[u] = paot.tile([P, GH * 512], _bf16, tag="aot", name="aot")
                if not causal:
                    mus[u] = pmu.tile([P, NT * 512], _bf16, tag="mu", name="mu")
                    nc.sync.dma_start(
                        mus[u][:].rearrange("p (t j) -> p t j", t=NT),
                        maskf[:, u * 512:(u + 1) * 512].rearrange(
                            "(t p) j -> p t j", t=NT))
                attn(u, 0)
                if u > 0:
                    wproj(u - 1, range(NT))
                for h in range(1, GH):
                    attn(u, h)
            wproj(NU - 1, range(NT))
    nc.compile()
    return nc


class _Runner:
    """Persistent PJRT executable for one compiled Bass module (SPMD over 8 cores)."""

    def __init__(self, nc, n_cores):
        import jax
        from jax.sharding import Mesh, PartitionSpec
        from jax.experimental.shard_map import shard_map
        from concourse.bass2jax import (
            _bass_exec_p, install_neuronx_cc_hook, partition_id_tensor)

        install_neuronx_cc_hook()
        self.jax = jax
        self.n_cores = n_cores
        partition_name = nc.partition_id_tensor.name if nc.partition_id_tensor else None
        in_names, out_names, out_avals = [], [], []
        for alloc in nc.m.functions[0].allocations:
            if not isinstance(alloc, mybir.MemoryLocationSet):
                continue
            name = alloc.memorylocations[0].name
            if alloc.kind == "ExternalInput":
                if name != partition_name:
                    in_names.append(name)
            elif alloc.kind == "ExternalOutput":
                out_names.append(name)
                out_avals.append(jax.core.ShapedArray(
                    tuple(alloc.tensor_shape), mybir.dt.np(alloc.dtype)))
        self.in_names, self.out_names, self.out_avals = in_names, out_names, out_avals
        n_params, n_outs = len(in_names), len(out_avals)
        all_in = list(in_names) + list(out_names)
        if partition_name is not None:
            all_in.append(partition_name)

        def _body(*args):
            operands = list(args)
            if partition_name is not None:
                operands.append(partition_id_tensor())
            return tuple(_bass_exec_p.bind(
                *operands,
                out_avals=tuple(out_avals), in_names=tuple(all_in),
                out_names=tuple(out_names), lowering_input_output_aliases=(),
                sim_require_finite=True, sim_require_nnan=True, nc=nc))

        devices = jax.devices()[:n_cores]
        mesh = Mesh(np.asarray(devices), ("core",))
        self.sharding = jax.sharding.NamedSharding(mesh, PartitionSpec("core"))
        self.fn = jax.jit(
            shard_map(_body, mesh=mesh,
                      in_specs=(PartitionSpec("core"),) * (n_params + n_outs),
                      out_specs=(PartitionSpec("core"),) * n_outs,
                      check_rep=False),
            keep_unused=True)
        self._dev_args = None

    def put_inputs(self, in_maps):
        jax = self.jax
        concat_in = [
            np.concatenate([np.asarray(in_maps[c][n]) for c in range(self.n_cores)], axis=0)
            for n in self.in_names]
        concat_zeros = [
            np.zeros((self.n_cores * a.shape[0], *a.shape[1:]), a.dtype)
            for a in self.out_avals]
        self._dev_args = [
            jax.device_put(v, self.sharding) for v in concat_in + concat_zeros]
        for a in self._dev_args:
            a.block_until_ready()

    def execute(self):
        return self.fn(*self._dev_args)

    def run(self, in_maps):
        last_err = None
        for attempt in range(3):
            try:
                self.put_inputs(in_maps)
                outs = self.execute()
                self.jax.block_until_ready(outs)
                return [
                    {n: np.asarray(outs[i]).reshape(
                        self.n_cores, *self.out_avals[i].shape)[c]
                     for i, n in enumerate(self.out_names)}
                    for c in range(self.n_cores)]
            except Exception as e:  # transient NRT faults: retry
                last_err = e
                import time
                time.sleep(2.0 * (attempt + 1))
        raise last_err


def _get_runner(causal: bool):
    if causal not in _cache:
        _cache[causal] = _Runner(_build(causal), NCORES)
    return _cache[causal]


def _host_prep(x, mask, Wq, Wk, Wv, Wo, causal):
    scale = np.float32(1.0) / np.sqrt(np.float32(HD))
    perm = np.concatenate(
        [np.concatenate([np.arange(0, HD, 2), np.arange(1, HD, 2)]) + HD * hh
         for hh in range(GH)])
    inv = (np.float32(1.0) / np.power(
        np.float32(10000.0),
        np.arange(0, HD, 2).astype(np.float32) / np.float32(HD))).astype(np.float32)
    ang = np.arange(S, dtype=np.float32)[:, None] * inv[None, :]
    cos_t = np.cos(ang).T.astype(np.float32)
    sin_t = np.sin(ang).T.astype(np.float32)
    cs_host = np.ascontiguousarray(np.concatenate([cos_t, cos_t], axis=0))
    ss_host = np.ascontiguousarray(np.concatenate([-sin_t, sin_t], axis=0))
    ones_host = np.ones((P, P), _bf)
    if causal:
        # keep-mask: tri[r, j] = 1 where key r <= query j (within block), else 0
        tri_host = np.triu(np.ones((P, P), np.float32)).astype(_bf)
    else:
        maskT = np.ascontiguousarray(mask.T).astype(_bf)
    xTs = [np.ascontiguousarray(x[b].T).astype(_bf) for b in range(B)]
    in_maps = []
    for c in range(NCORES):
        b, g = c // G, c % G
        rows = slice(g * GD, (g + 1) * GD)
        m = {
            "xT": xTs[b],
            "wq": np.ascontiguousarray(Wq[rows].T[:, perm] * scale).astype(_bf),
            "wk": np.ascontiguousarray(Wk[rows].T[:, perm]).astype(_bf),
            "wv": np.ascontiguousarray(Wv[rows].T).astype(_bf),
            "wo": np.ascontiguousarray(Wo[:, rows].T).astype(_bf),
            "cs": cs_host,
            "ss": ss_host,
            "ones": ones_host,
        }
        if causal:
            m["tri"] = tri_host
        else:
            m["maskf"] = maskT
        in_maps.append(m)
    return in_maps


def kernel(x, mask, Wq, Wk, Wv, Wo):
    x = np.asarray(x, dtype=np.float32)
    mask = np.asarray(mask, dtype=np.float32)
    Wq = np.asarray(Wq, dtype=np.float32)
    Wk = np.asarray(Wk, dtype=np.float32)
    Wv = np.asarray(Wv, dtype=np.float32)
    Wo = np.asarray(Wo, dtype=np.float32)
    expected_mask = np.triu(np.full((S, S), -1e9, dtype=np.float32), k=1)
    causal = bool(np.array_equal(mask, expected_mask))
    runner = _get_runner(causal)
    in_maps = _host_prep(x, mask, Wq, Wk, Wv, Wo, causal)
    results = runner.run(in_maps)
    out = np.empty((B, S, D), np.float32)
    for b in range(B):
        acc = results[b * G]["outT"].astype(np.float32)
        for g in range(1, G):
            acc += results[b * G + g]["outT"].astype(np.float32)
        out[b] = acc.T
    return out
